# revision 34
# baseline (speedup 1.0000x reference)
"""Distributed Trainium2 Bass kernel for causal multi-head attention.

Module:  qkv = x @ w_qkv + b_qkv ; causal softmax attention (16 heads, d=64);
         out = z @ w_out + b_out.   x: [4, 2048, 1024] f32.

Sharding (8 NeuronCores): core c handles batch b = c//2 and head-group
hg = c%2 (8 of 16 heads).  Each core computes its heads' Q/K/V projections,
causal flash attention, and a partial out-projection over its 512 head-dims.
The two cores sharing a batch each return a partial out^T [1024, 2048]; the
host sums the pair and transposes (tensor-parallel reduce done host-side —
a 2-rank on-device all-reduce of 8MB would cost more than the whole kernel).

Compute is bf16 on the TensorEngine with f32 PSUM accumulation
(fp32 matmul is 4x slower on TRN2; measured end-to-end rel err ~6e-3).

Layout choices (all transposes are free, host-side numpy):
- x arrives transposed per core: xt [128, 8, 2048] bf16 so the QKV
  projection needs no on-device transpose.
- Q and K are produced feature-major (qT/kT [128, 4, 2048]: partition tile p
  holds head pair (2p, 2p+1); partitions 0-63 = head 2p, 64-127 = head 2p+1).
  Scores are computed transposed, S^T = K-stationary matmul, as two
  concurrent row-group matmuls (K=64 contraction at base partitions 0/64).
- V is produced token-major [tokens, 64] per head with a ones-column
  appended: the PV matmul (M=65) yields z^T rows AND the softmax
  denominator r[q] = sum_k exp(s) in PSUM row 64 — no cross-partition
  reduction anywhere.
- 1/sqrt(head_dim) is folded into the K projection weights host-side.
- softmax skips max-subtraction (logits/8 here are << f32 exp overflow);
  exp is restricted to the causal span and diagonal 128-blocks get a
  triangular 0/1 mask multiplicatively after exp.
- softmax reciprocals are spread over 16 partitions via a DRAM bounce
  (a [1,1024] DVE reciprocal runs on one lane at 8 cyc/elem = 8.5us;
  [16,64] takes 0.5us and only 16 DMA descriptors), the inverse is
  broadcast back across partitions with a stride-0 DRAM-read DMA, and
  each unit's normalize is deferred (z evicted to SBUF eagerly) so the
  in-order TensorEngine never waits on the round-trip.
- the tensor-engine schedule is micro-woven: the next token-chunk's
  projections and the finished query-tiles' out-projections are emitted
  <=2 matmuls per attention ktile iteration, filling the PE slots that
  attention's ACT-paced exp pipeline would otherwise leave idle.  The
  first projection runs kd-outer over 4 concurrent PSUM groups so the PE
  ramps with the input DMA, and the final out-projection accumulates
  head-pairs 0-2 while the last softmax round-trip is in flight.
"""

import sys
import types

import numpy as np
import ml_dtypes

# ── NTFF profile hook shim: the container's antenv stub lacks axon_hooks, so
# trn_boot's hook registration degraded silently.  Recreate it so that
# trace=True (or BASS_TRACE=1) can report HW exec time. ──
import antenv

if "antenv.axon_hooks" not in sys.modules:
    _m = types.ModuleType("antenv.axon_hooks")
    _m._hook = None
    _m.set_axon_ntff_profile_hook = lambda h: setattr(_m, "_hook", h)
    _m.get_axon_ntff_profile_hook = lambda: _m._hook
    sys.modules["antenv.axon_hooks"] = _m
    antenv.axon_hooks = _m
    try:
        from trn_agent_boot.trn_boot import _ntff_profile_via_ctypes

        _m.set_axon_ntff_profile_hook(
            _ntff_profile_via_ctypes("/opt/axon/libaxon_pjrt.so")
        )
    except Exception:
        pass

import concourse.bass as bass
import concourse.mybir as mybir
import concourse.tile as tile
from concourse import bacc, bass_utils
from concourse.bass_utils import run_bass_kernel_spmd

# fishnet artifact upload is unavailable here; keep the trace path local.
bass_utils.upload_artifacts = lambda tmpdir: "local://" + str(tmpdir)

BF = ml_dtypes.bfloat16
F32 = mybir.dt.float32
BF16 = mybir.dt.bfloat16
FN = mybir.ActivationFunctionType
MUL = mybir.AluOpType.mult

P = 128
S = 2048          # sequence length
D = 1024          # d_model
HD = 64           # head dim
N_CORES = 8
LOC_H = 8         # heads per core
NPAIR = 4         # head pairs per core
NQT = 4           # query tiles of 512
QW = 512          # query tile width
NKT = 16          # key tiles of 128
KD = 8            # D / 128 contraction tiles
FQKV = 3 * LOC_H * HD   # 1536 local qkv features
HDL = LOC_H * HD        # 512 local head dims

TRACE = False
LAST_RESULT = None   # BassKernelResults of the most recent run (for test.py)

_NC_CACHE = {}


def build_nc(qk_bias_nz: bool, v_bias_nz: bool, out_bias_nz: bool):
    nc = bacc.Bacc()
    xt_d = nc.dram_tensor("xt", [P, KD, S], BF16, kind="ExternalInput")
    wqkv_d = nc.dram_tensor("wqkv", [P, KD, FQKV], BF16, kind="ExternalInput")
    bqkv_d = nc.dram_tensor("bqkv", [P, 12], F32, kind="ExternalInput")
    wout_d = nc.dram_tensor("wout", [P, 4, D], BF16, kind="ExternalInput")
    bout_d = nc.dram_tensor("bout", [P, 8], F32, kind="ExternalInput")
    out_d = nc.dram_tensor("out", [D, S], F32, kind="ExternalOutput")

    with tile.TileContext(nc) as tc:
        with tc.tile_pool(name="const", bufs=1) as const, \
             tc.tile_pool(name="work", bufs=2) as work, \
             tc.tile_pool(name="work4", bufs=4) as work4, \
             tc.tile_pool(name="osbp", bufs=3) as osbp, \
             tc.tile_pool(name="upool", bufs=7) as upool, \
             tc.tile_pool(name="dram", bufs=4, space="DRAM") as dram, \
             tc.tile_pool(name="ps_s", bufs=2, space="PSUM") as ps_s, \
             tc.tile_pool(name="ps_z", bufs=2, space="PSUM") as ps_z:

            # ---- constant loads, split into ~512-col pieces across the 16
            # DMA queues (a single queue moves ~31 GB/s; one whole kd-chunk
            # on one queue would gate the first matmuls by ~14us) ----
            xt_sb = const.tile([P, KD, S], BF16, tag="xt")
            wqkv_sb = const.tile([P, KD, FQKV], BF16, tag="wqkv")
            for kd in range(KD):
                nc.sync.dma_start(wqkv_sb[:, kd, 0:QW], wqkv_d[:, kd, 0:QW])
                for j in range(0, S // 2, QW):
                    nc.sync.dma_start(xt_sb[:, kd, j : j + QW],
                                      xt_d[:, kd, j : j + QW])
            for kd in range(KD):
                nc.sync.dma_start(wqkv_sb[:, kd, QW : 2 * QW],
                                  wqkv_d[:, kd, QW : 2 * QW])
                for j in range(S // 2, S, QW):
                    nc.sync.dma_start(xt_sb[:, kd, j : j + QW],
                                      xt_d[:, kd, j : j + QW])
            for kd in range(KD):
                nc.sync.dma_start(wqkv_sb[:, kd, 2 * QW : FQKV],
                                  wqkv_d[:, kd, 2 * QW : FQKV])
            wout_sb = const.tile([P, 4, D], BF16, tag="wout")
            nc.sync.dma_start(wout_sb[:], wout_d[:])
            bqkv_sb = const.tile([P, 12], F32, tag="bqkv")
            nc.sync.dma_start(bqkv_sb[:], bqkv_d[:])
            bout_sb = const.tile([P, 8], F32, tag="bout")
            nc.sync.dma_start(bout_sb[:], bout_d[:])

            qT = const.tile([P, NPAIR, S], BF16, tag="qT")
            kT = const.tile([P, NPAIR, S], BF16, tag="kT")
            zT = const.tile([P, 4, S], BF16, tag="zT")
            v_sb = const.tile([P, LOC_H, NKT, HD + 1], BF16, tag="v")
            nc.vector.memset(v_sb[:, :, :, HD : HD + 1], 1.0)

            # triangular 0/1 mask (keep iff k <= q) for diagonal 128-blocks
            tri = const.tile([P, P], BF16, tag="tri")
            nc.gpsimd.memset(tri[:], 1.0)
            nc.gpsimd.affine_select(
                out=tri[:], in_=tri[:],
                compare_op=mybir.AluOpType.is_ge,
                fill=0.0, base=0,
                pattern=[[1, P]], channel_multiplier=-1,
            )
            ones1 = const.tile([1, HD], BF16, tag="ones1")
            nc.vector.memset(ones1[:], 1.0)
            ones11 = const.tile([65, 1], BF16, tag="ones11")
            nc.vector.memset(ones11[:], 1.0)

            if v_bias_nz:
                # broadcast the v-bias (free axis) across partitions via matmul
                bv_bf = const.tile([1, HDL], BF16, tag="bvbf")
                bvrow = const.tile([1, HDL], F32, tag="bvrow")
                for j in range(4):
                    nc.sync.dma_start(
                        bvrow[0:1, j * P : (j + 1) * P],
                        bqkv_sb[:, 8 + j : 9 + j].rearrange("p one -> one p"),
                    )
                nc.vector.tensor_copy(bv_bf[:], bvrow[:])
                ones128 = const.tile([1, P], BF16, tag="ones128")
                nc.vector.memset(ones128[:], 1.0)
                ps_bv = ps_s.tile([P, 2 * QW], F32, tag="s")
                nc.tensor.matmul(ps_bv[:, :HDL], ones128[:], bv_bf[:],
                                 start=True, stop=True)
                bv_bc = const.tile([P, HDL], F32, tag="bvbc")
                nc.vector.tensor_copy(bv_bc[:], ps_bv[:, :HDL])

            def qk_copy(dst_ap, ps_ap, bias_ap):
                if qk_bias_nz:
                    nc.vector.tensor_scalar_add(dst_ap, ps_ap, bias_ap)
                else:
                    nc.vector.tensor_copy(dst_ap, ps_ap)

            def alt_ps(i):
                """Alternate psum allocations between the two pools (4-deep
                rotation) so PSUM->SBUF copies never gate the next group."""
                pool, tag = ((ps_s, "s"), (ps_z, "z"))[i % 2]
                return pool.tile([P, 2 * QW], F32, tag=tag, name=f"ps_{tag}")

            _alt = [0]

            def fill_ps(state):
                if "ps" not in state:
                    pool, tag = state.get("pt", (ps_z, "z"))
                    state["ps"] = pool.tile([P, 2 * QW], F32, tag=tag,
                                            name="ps_fill")
                return state["ps"]

            def proj_group(tcA, fo, pt=None):
                """One feature-tile of Q/K projection for chunks tcA, tcA+1,
                as a list of micro-steps (matmuls + final copy)."""
                tok2 = slice(tcA * QW, (tcA + 2) * QW)
                fsl = slice(fo * P, (fo + 1) * P)
                state = {} if pt is None else {"pt": pt}
                steps = []

                def mk(kd, h):
                    def f():
                        ps = fill_ps(state)
                        nc.tensor.matmul(
                            ps[:, h * QW : (h + 1) * QW], wqkv_sb[:, kd, fsl],
                            xt_sb[:, kd, (tcA + h) * QW : (tcA + h + 1) * QW],
                            start=(kd == 0), stop=(kd == KD - 1),
                        )
                    return f

                for kd in range(KD):
                    steps.append(mk(kd, 0))
                    steps.append(mk(kd, 1))

                def fin():
                    ps = state["ps"]
                    if fo < 4:
                        qk_copy(qT[:, fo, tok2], ps[:], bqkv_sb[:, fo : fo + 1])
                    else:
                        qk_copy(kT[:, fo - 4, tok2], ps[:],
                                bqkv_sb[:, fo : fo + 1])

                steps.append(fin)
                return steps

            def v_group(tp, pt=None):
                """V projection for token tiles tp, tp+1 (token-major)."""
                state = {} if pt is None else {"pt": pt}
                steps = []

                def mk(kd, h):
                    def f():
                        ps = fill_ps(state)
                        nc.tensor.matmul(
                            ps[:, h * QW : (h + 1) * QW],
                            xt_sb[:, kd, (tp + h) * P : (tp + h + 1) * P],
                            wqkv_sb[:, kd, 2 * HDL : 3 * HDL],
                            start=(kd == 0), stop=(kd == KD - 1),
                        )
                    return f

                for kd in range(KD):
                    steps.append(mk(kd, 0))
                    steps.append(mk(kd, 1))

                def fin():
                    ps = state["ps"]
                    psv = ps[:].rearrange("p (t h d) -> p h t d", t=2, d=HD)
                    if v_bias_nz:
                        nc.vector.tensor_tensor(
                            v_sb[:, :, tp : tp + 2, 0:HD], psv,
                            bv_bc[:].rearrange("p (h d) -> p h d", d=HD)[
                                :, :, None, :
                            ].to_broadcast((P, LOC_H, 2, HD)),
                            mybir.AluOpType.add,
                        )
                    else:
                        nc.vector.tensor_copy(v_sb[:, :, tp : tp + 2, 0:HD], psv)

                steps.append(fin)
                return steps

            def proj_groups(tcA):
                return [proj_group(tcA, fo) for fo in range(8)] + [
                    v_group(tp) for tp in range(4 * tcA, 4 * (tcA + 2), 2)
                ]

            pend = []   # deferred normalize closures (keep <= 1)

            def attn_unit(qt, p_i, fast_rt=False, rt_eng=None):
                dmae = rt_eng or nc.sync
                nkt = 4 * (qt + 1)
                qs = slice(qt * QW, (qt + 1) * QW)
                psZ = ps_z.tile([P, 2 * QW], F32, tag="z")
                u_tiles = [None] * nkt

                def av(kt):
                    # diagonal tiles only touch queries >= m*128 (causal)
                    m = kt - 4 * qt
                    o = m * P if m > 0 else 0
                    first, last = (kt == 0), (kt == nkt - 1)
                    nc.tensor.matmul(
                        psZ[0 : HD + 1, o:QW], v_sb[:, 2 * p_i, kt, :],
                        u_tiles[kt][:, o:QW],
                        start=first, stop=last, skip_group_check=True,
                    )
                    nc.tensor.matmul(
                        psZ[0 : HD + 1, QW + o : 2 * QW], v_sb[:, 2 * p_i + 1, kt, :],
                        u_tiles[kt][:, QW + o : 2 * QW],
                        start=first, stop=last, skip_group_check=True,
                    )

                for kt in range(nkt):
                    ks = slice(kt * P, (kt + 1) * P)
                    m = kt - 4 * qt
                    o = m * P if m > 0 else 0
                    psS = ps_s.tile([P, 2 * QW], F32, tag="s")
                    nc.tensor.matmul(psS[:, o:QW], kT[0:64, p_i, ks],
                                     qT[0:64, p_i, qs][:, o:QW],
                                     start=True, stop=True)
                    nc.tensor.matmul(psS[:, QW + o : 2 * QW], kT[64:128, p_i, ks],
                                     qT[64:128, p_i, qs][:, o:QW],
                                     start=True, stop=True)
                    u = upool.tile([P, 2 * QW], BF16, tag="U")
                    u_tiles[kt] = u
                    if m < 0:
                        nc.scalar.activation(u[:], psS[:], FN.Exp)
                    else:
                        uv = u[:].rearrange("p (h q) -> p h q", h=2)
                        pv = psS[:].rearrange("p (h q) -> p h q", h=2)
                        nc.scalar.activation(
                            uv[:, :, o:QW], pv[:, :, o:QW], FN.Exp
                        )
                        blk = slice(o, o + P)
                        nc.vector.tensor_tensor(
                            uv[:, :, blk], uv[:, :, blk],
                            tri[:, None, :].to_broadcast((P, 2, P)), MUL,
                        )
                    if kt >= 2:
                        av(kt - 2)
                    pump(2)
                av(nkt - 2)
                av(nkt - 1)
                close_open_group()

                # part 1: evict z to SBUF (frees the PSUM bank), extract the
                # denominators, spread over 128 partitions, reciprocal, and
                # land in DRAM as [1024] bf16 rinv (A-half | B-half).
                # fast_rt (used for the last unit, where the latency is bare):
                # spread via tiny PE matmuls (r-row as weights) instead of the
                # two extra DMA hops through DRAM.
                if fast_rt:
                    stb = work4.tile([65, 2 * QW], BF16, tag="rstb")
                    nc.vector.tensor_copy(stb[64:65, :], psZ[64:65, :])
                    z_st = work4.tile([HD, 2 * QW], F32, tag="zst")
                    nc.vector.tensor_copy(z_st[:], psZ[0:HD, :])
                    psR2 = ps_s.tile([P, 2 * QW], F32, tag="s", name="ps_spread")
                    for j in range(8):
                        nc.tensor.matmul(psR2[:, j : j + 1],
                                         stb[64:65, j * P : (j + 1) * P],
                                         ones11[64:65, :], start=True, stop=True)
                    rspo = work4.tile([P, 8], BF16, tag="rspo")
                    with nc.allow_low_precision(reason="bf16 softmax denom"):
                        nc.vector.reciprocal(rspo[:], psR2[:, 0:8])
                    d2 = dram.tile([1, 2 * QW], BF16, tag="d2")
                    nc.sync.dma_start(
                        d2[:].rearrange("one (f p) -> (one p) f", p=P), rspo[:]
                    )
                else:
                    st = work4.tile([65, 2 * QW], F32, tag="rst")
                    nc.vector.tensor_copy(st[64:65, :], psZ[64:65, :])
                    z_st = work4.tile([HD, 2 * QW], F32, tag="zst")
                    nc.vector.tensor_copy(z_st[:], psZ[0:HD, :])
                    d1 = dram.tile([1, 2 * QW], F32, tag="d1")
                    dmae.dma_start(d1[:], st[64:65, :])
                    rsp = work4.tile([16, HD], F32, tag="rsp")
                    dmae.dma_start(
                        rsp[:], d1[:].rearrange("one (p f) -> (one p) f", p=16)
                    )
                    rspo = work4.tile([16, HD], BF16, tag="rspo")
                    with nc.allow_low_precision(reason="bf16 softmax denom"):
                        nc.vector.reciprocal(rspo[:], rsp[:])
                    d2 = dram.tile([1, 2 * QW], BF16, tag="d2")
                    dmae.dma_start(
                        d2[:].rearrange("one (p f) -> (one p) f", p=16), rspo[:]
                    )

                def part2():
                    # broadcast 1/r across partitions straight from DRAM
                    rb = work.tile([HD, 2 * QW], BF16, tag="rb")
                    for q in range(4):
                        dmae.dma_start(
                            rb[q * 16 : (q + 1) * 16, :],
                            d2[0:1, :].to_broadcast((16, 2 * QW)),
                        )
                    nc.vector.tensor_tensor(
                        zT[0:HD, p_i, qs], z_st[:, 0:QW], rb[:, 0:QW], MUL
                    )
                    stB = work.tile([HD, QW], BF16, tag="stB")
                    nc.vector.tensor_tensor(
                        stB[:], z_st[:, QW : 2 * QW], rb[:, QW : 2 * QW], MUL
                    )
                    dmae.dma_start(zT[64:128, p_i, qs], stB[:])

                return part2

            fillers = []      # list of group step-lists; groups[0] may be open
            fill_open = [False]

            def pump(n):
                done = 0
                while done < n and fillers:
                    g = fillers[0]
                    g.pop(0)()
                    fill_open[0] = True
                    done += 1
                    if not g:
                        fillers.pop(0)
                        fill_open[0] = False

            def close_open_group():
                if fillers and fill_open[0]:
                    g = fillers.pop(0)
                    for f in g:
                        f()
                    fill_open[0] = False

            def attn(qt, depth=3, fast_last=False):
                for p_i in range(NPAIR):
                    p2 = attn_unit(qt, p_i, fast_rt=(fast_last and p_i == 3),
                                   rt_eng=nc.scalar if (qt == 3 and p_i == 3)
                                   else None)
                    pend.append(p2)
                    while len(pend) > depth:
                        pend.pop(0)()

            def flush():
                close_open_group()
                while pend or fillers:
                    if pend:
                        pend.pop(0)()
                    if fillers:
                        for f in fillers.pop(0):
                            f()

            out_r = out_d[:].rearrange("(mo p) t -> p mo t", p=P)

            def op_group(qtA, mo, two):
                """out-projection group: query tiles qtA(,qtA+1), one mo tile."""
                w = 2 * QW if two else QW
                qs2 = slice(qtA * QW, qtA * QW + w)
                msl = slice(mo * P, (mo + 1) * P)
                state = {}
                steps = []

                def mk(ko, h):
                    def f():
                        psO = fill_ps(state)
                        qs = slice((qtA + h) * QW, (qtA + h + 1) * QW)
                        nc.tensor.matmul(psO[:, h * QW : (h + 1) * QW],
                                         wout_sb[:, ko, msl], zT[:, ko, qs],
                                         start=(ko == 0), stop=(ko == 3))
                    return f

                for ko in range(4):
                    for h in range(2 if two else 1):
                        steps.append(mk(ko, h))

                def fin():
                    psO = state["ps"]
                    osb = osbp.tile([P, 2 * QW], F32, tag="osb")
                    if out_bias_nz:
                        nc.vector.tensor_scalar_add(osb[:, 0:w], psO[:, 0:w],
                                                    bout_sb[:, mo : mo + 1])
                    else:
                        nc.vector.tensor_copy(osb[:, 0:w], psO[:, 0:w])
                    nc.gpsimd.dma_start(out_r[:, mo, qs2], osb[:, 0:w])

                steps.append(fin)
                return steps

            def op_groups(qtA, two=True):
                return [op_group(qtA, mo, two) for mo in range(8)]

            # ---- schedule: later projection / out-projection groups are
            # micro-woven into attention (<=2 filler matmuls per ktile
            # iteration: attention is ACT-paced, the fillers use the idle
            # TensorEngine slots without starving the exp pipeline) ----
            # proj01 runs kd-outer in batches of 4 concurrent psum groups so
            # the first matmuls only need the first input DMA pieces (the PE
            # ramps with the loads instead of idling ~20us).
            pts = [(ps_s, "s"), (ps_s, "s"), (ps_z, "z"), (ps_z, "z")]
            p01_sets = [
                [proj_group(0, fo, pts[i]) for i, fo in enumerate(range(0, 4))],
                [proj_group(0, fo, pts[i]) for i, fo in enumerate(range(4, 8))],
                [v_group(tp, pts[i]) for i, tp in enumerate(range(0, 8, 2))],
            ]
            for gset in p01_sets:
                n = len(gset[0]) - 1
                for i in range(n):
                    for g in gset:
                        g[i]()
                for g in gset:
                    g[n]()
            fillers.extend(proj_groups(2))       # 12 groups
            attn(0)
            attn(1)
            flush()                               # qt0/qt1 normalizes done
            fillers.extend(op_groups(0, two=True))
            attn(2)
            flush()
            op2 = op_groups(2, two=False)
            fillers.extend(op2[:2])
            attn(3, depth=1)
            close_open_group()
            for g in fillers:
                for f in g:
                    f()
            fillers.clear()
            for g in op2[2:]:
                for f in g:
                    f()
            # tail: compute out-proj(qt3) contributions of pairs 0-2 while the
            # final unit's reciprocal round-trip is in flight, then finish
            # with pair 3 once its normalize lands.
            qs3 = slice(3 * QW, 4 * QW)
            tails = []
            for mh in range(4):
                psO = pts[mh][0].tile([P, 2 * QW], F32, tag=pts[mh][1],
                                      name="ps_tail")
                for h in range(2):
                    mo = 2 * mh + h
                    msl = slice(mo * P, (mo + 1) * P)
                    for ko in range(3):
                        nc.tensor.matmul(psO[:, h * QW : (h + 1) * QW],
                                         wout_sb[:, ko, msl], zT[:, ko, qs3],
                                         start=(ko == 0), stop=False,
                                         skip_group_check=True)
                tails.append(psO)
            flush()   # part2(qt3, p3)
            for mh in range(4):
                psO = tails[mh]
                for h in range(2):
                    mo = 2 * mh + h
                    msl = slice(mo * P, (mo + 1) * P)
                    nc.tensor.matmul(psO[:, h * QW : (h + 1) * QW],
                                     wout_sb[:, 3, msl], zT[:, 3, qs3],
                                     start=False, stop=True,
                                     skip_group_check=True)
                osb = osbp.tile([P, 2 * QW], F32, tag="osb")
                if out_bias_nz:
                    for h in range(2):
                        mo = 2 * mh + h
                        nc.vector.tensor_scalar_add(
                            osb[:, h * QW : (h + 1) * QW],
                            psO[:, h * QW : (h + 1) * QW],
                            bout_sb[:, mo : mo + 1])
                elif mh % 2 == 0:
                    nc.vector.tensor_copy(osb[:], psO[:])
                else:
                    nc.scalar.activation(osb[:], psO[:], FN.Copy)
                for h in range(2):
                    mo = 2 * mh + h
                    nc.sync.dma_start(out_r[:, mo, qs3],
                                      osb[:, h * QW : (h + 1) * QW])

    nc.finalize()
    return nc


def _tile_p(a, inner):
    """[n*128, m...] -> [128, n, m...] partition-major, contiguous."""
    n = a.shape[0] // P
    return np.ascontiguousarray(
        a.reshape(n, P, *a.shape[1:]).transpose(1, 0, *range(2, a.ndim + 1))
    )


def kernel(x, w_qkv, b_qkv, w_out, b_out):
    global LAST_RESULT
    x = np.asarray(x)
    w_qkv = np.asarray(w_qkv, dtype=np.float32)
    b_qkv = np.asarray(b_qkv, dtype=np.float32)
    w_out = np.asarray(w_out, dtype=np.float32)
    b_out = np.asarray(b_out, dtype=np.float32)
    B = x.shape[0]

    in_maps = []
    qk_bias_nz = bool(np.any(b_qkv[: 2 * D] != 0.0))
    v_bias_nz = bool(np.any(b_qkv[2 * D :] != 0.0))
    out_bias_nz = bool(np.any(b_out != 0.0))
    for c in range(N_CORES):
        b = c // 2
        hg = c % 2
        heads = range(hg * LOC_H, (hg + 1) * LOC_H)
        cols = np.array(
            [sec * D + h * HD + j for sec in range(3) for h in heads
             for j in range(HD)]
        )
        w_loc = w_qkv[:, cols].copy()
        w_loc[:, HDL : 2 * HDL] *= 1.0 / np.sqrt(HD)
        b_loc = b_qkv[cols].copy()
        b_loc[HDL : 2 * HDL] *= 1.0 / np.sqrt(HD)
        bo = b_out if hg == 0 else np.zeros_like(b_out)
        xt = np.ascontiguousarray(x[b].T)
        in_maps.append(
            dict(
                xt=_tile_p(xt.astype(BF), KD),
                wqkv=_tile_p(w_loc.astype(BF), KD),
                bqkv=np.ascontiguousarray(b_loc.reshape(12, P).T),
                wout=_tile_p(w_out[cols[2 * HDL :] - 2 * D, :].astype(BF), 4),
                bout=np.ascontiguousarray(bo.reshape(8, P).T),
            )
        )

    key = (qk_bias_nz, v_bias_nz, out_bias_nz)
    if key not in _NC_CACHE:
        _NC_CACHE[key] = build_nc(*key)
    nc = _NC_CACHE[key]

    res = run_bass_kernel_spmd(
        nc, in_maps, core_ids=list(range(N_CORES)), trace=TRACE
    )
    LAST_RESULT = res

    out = np.empty((B, S, D), dtype=np.float32)
    for b in range(B):
        out[b] = (res.results[2 * b]["out"] + res.results[2 * b + 1]["out"]).T
    return out


# revision 35
# speedup vs baseline: 1.1683x; 1.1683x over previous
"""Distributed Trainium2 Bass kernel for causal multi-head attention.

Module:  qkv = x @ w_qkv + b_qkv ; causal softmax attention (16 heads, d=64);
         out = z @ w_out + b_out.   x: [4, 2048, 1024] f32.

Sharding (8 NeuronCores): core c handles batch b = c//2 and head-group
hg = c%2 (8 of 16 heads).  Each core computes its heads' Q/K/V projections,
causal flash attention, and a partial out-projection over its 512 head-dims.
The two cores sharing a batch each return a partial out^T [1024, 2048]; the
host sums the pair and transposes (tensor-parallel reduce done host-side —
a 2-rank on-device all-reduce of 8MB would cost more than the whole kernel).

Compute is bf16 on the TensorEngine with f32 PSUM accumulation
(fp32 matmul is 4x slower on TRN2; measured end-to-end rel err ~6e-3).

Layout choices (all transposes are free, host-side numpy):
- x arrives transposed per core: xt [128, 8, 2048] bf16 so the QKV
  projection needs no on-device transpose.
- Q and K are produced feature-major (qT/kT [128, 4, 2048]: partition tile p
  holds head pair (2p, 2p+1); partitions 0-63 = head 2p, 64-127 = head 2p+1).
  Scores are computed transposed, S^T = K-stationary matmul, as two
  concurrent row-group matmuls (K=64 contraction at base partitions 0/64).
- V is produced token-major [tokens, 64] per head with a ones-column
  appended: the PV matmul (M=65) yields z^T rows AND the softmax
  denominator r[q] = sum_k exp(s) in PSUM row 64 — no cross-partition
  reduction anywhere.
- 1/sqrt(head_dim) is folded into the K projection weights host-side.
- softmax skips max-subtraction (logits/8 here are << f32 exp overflow);
  exp is restricted to the causal span and diagonal 128-blocks get a
  triangular 0/1 mask multiplicatively after exp.
- softmax reciprocals are spread over 16 partitions via a DRAM bounce
  (a [1,1024] DVE reciprocal runs on one lane at 8 cyc/elem = 8.5us;
  [16,64] takes 0.5us and only 16 DMA descriptors), the inverse is
  broadcast back across partitions with a stride-0 DRAM-read DMA, and
  each unit's normalize is deferred (z evicted to SBUF eagerly) so the
  in-order TensorEngine never waits on the round-trip.
- the tensor-engine schedule is micro-woven: the next token-chunk's
  projections and the finished query-tiles' out-projections are emitted
  <=2 matmuls per attention ktile iteration, filling the PE slots that
  attention's ACT-paced exp pipeline would otherwise leave idle.  The
  first projection runs kd-outer over 4 concurrent PSUM groups so the PE
  ramps with the input DMA, and the final out-projection accumulates
  head-pairs 0-2 while the last softmax round-trip is in flight.
"""

import sys
import types

import numpy as np
import ml_dtypes

# ── NTFF profile hook shim: the container's antenv stub lacks axon_hooks, so
# trn_boot's hook registration degraded silently.  Recreate it so that
# trace=True (or BASS_TRACE=1) can report HW exec time. ──
import antenv

if "antenv.axon_hooks" not in sys.modules:
    _m = types.ModuleType("antenv.axon_hooks")
    _m._hook = None
    _m.set_axon_ntff_profile_hook = lambda h: setattr(_m, "_hook", h)
    _m.get_axon_ntff_profile_hook = lambda: _m._hook
    sys.modules["antenv.axon_hooks"] = _m
    antenv.axon_hooks = _m
    try:
        from trn_agent_boot.trn_boot import _ntff_profile_via_ctypes

        _m.set_axon_ntff_profile_hook(
            _ntff_profile_via_ctypes("/opt/axon/libaxon_pjrt.so")
        )
    except Exception:
        pass

import concourse.bass as bass
import concourse.mybir as mybir
import concourse.tile as tile
from concourse import bacc, bass_utils
from concourse.bass_utils import run_bass_kernel_spmd

# fishnet artifact upload is unavailable here; keep the trace path local.
bass_utils.upload_artifacts = lambda tmpdir: "local://" + str(tmpdir)

BF = ml_dtypes.bfloat16
F32 = mybir.dt.float32
BF16 = mybir.dt.bfloat16
FN = mybir.ActivationFunctionType
MUL = mybir.AluOpType.mult

P = 128
S = 2048          # sequence length
D = 1024          # d_model
HD = 64           # head dim
N_CORES = 8
LOC_H = 8         # heads per core
NPAIR = 4         # head pairs per core
NQT = 4           # query tiles of 512
QW = 512          # query tile width
NKT = 16          # key tiles of 128
KD = 8            # D / 128 contraction tiles
FQKV = 3 * LOC_H * HD   # 1536 local qkv features
HDL = LOC_H * HD        # 512 local head dims

TRACE = False
LAST_RESULT = None   # BassKernelResults of the most recent run (for test.py)

_NC_CACHE = {}


def build_nc(qk_bias_nz: bool, v_bias_nz: bool, out_bias_nz: bool):
    nc = bacc.Bacc()
    xt_d = nc.dram_tensor("xt", [P, KD, S], BF16, kind="ExternalInput")
    wqkv_d = nc.dram_tensor("wqkv", [P, KD, FQKV], BF16, kind="ExternalInput")
    bqkv_d = nc.dram_tensor("bqkv", [P, 12], F32, kind="ExternalInput")
    wout_d = nc.dram_tensor("wout", [P, 4, D], BF16, kind="ExternalInput")
    bout_d = nc.dram_tensor("bout", [P, 8], F32, kind="ExternalInput")
    out_d = nc.dram_tensor("out", [D, S], F32, kind="ExternalOutput")

    with tile.TileContext(nc) as tc:
        with tc.tile_pool(name="const", bufs=1) as const, \
             tc.tile_pool(name="work", bufs=2) as work, \
             tc.tile_pool(name="work4", bufs=4) as work4, \
             tc.tile_pool(name="upool", bufs=7) as upool, \
             tc.tile_pool(name="dram", bufs=4, space="DRAM") as dram, \
             tc.tile_pool(name="ps_s", bufs=2, space="PSUM") as ps_s, \
             tc.tile_pool(name="ps_z", bufs=2, space="PSUM") as ps_z:

            # ---- constant loads, split into ~512-col pieces across the 16
            # DMA queues (a single queue moves ~31 GB/s; one whole kd-chunk
            # on one queue would gate the first matmuls by ~14us) ----
            xt_sb = const.tile([P, KD, S], BF16, tag="xt")
            wqkv_sb = const.tile([P, KD, FQKV], BF16, tag="wqkv")
            for kd in range(KD):
                nc.sync.dma_start(wqkv_sb[:, kd, 0:QW], wqkv_d[:, kd, 0:QW])
                for j in range(0, S // 2, QW):
                    nc.sync.dma_start(xt_sb[:, kd, j : j + QW],
                                      xt_d[:, kd, j : j + QW])
            for kd in range(KD):
                nc.sync.dma_start(wqkv_sb[:, kd, QW : 2 * QW],
                                  wqkv_d[:, kd, QW : 2 * QW])
                for j in range(S // 2, S, QW):
                    nc.sync.dma_start(xt_sb[:, kd, j : j + QW],
                                      xt_d[:, kd, j : j + QW])
            for kd in range(KD):
                nc.sync.dma_start(wqkv_sb[:, kd, 2 * QW : FQKV],
                                  wqkv_d[:, kd, 2 * QW : FQKV])
            wout_sb = const.tile([P, 4, D], BF16, tag="wout")
            nc.sync.dma_start(wout_sb[:], wout_d[:])
            bqkv_sb = const.tile([P, 12], F32, tag="bqkv")
            nc.sync.dma_start(bqkv_sb[:], bqkv_d[:])
            bout_sb = const.tile([P, 8], F32, tag="bout")
            nc.sync.dma_start(bout_sb[:], bout_d[:])

            qT = const.tile([P, NPAIR, S], BF16, tag="qT")
            kT = const.tile([P, NPAIR, S], BF16, tag="kT")
            zT = const.tile([P, 4, S], BF16, tag="zT")
            v_sb = const.tile([P, LOC_H, NKT, HD + 1], BF16, tag="v")
            nc.vector.memset(v_sb[:, :, :, HD : HD + 1], 1.0)

            # triangular 0/1 mask (keep iff k <= q) for diagonal 128-blocks
            tri = const.tile([P, P], BF16, tag="tri")
            nc.gpsimd.memset(tri[:], 1.0)
            nc.gpsimd.affine_select(
                out=tri[:], in_=tri[:],
                compare_op=mybir.AluOpType.is_ge,
                fill=0.0, base=0,
                pattern=[[1, P]], channel_multiplier=-1,
            )
            ones1 = const.tile([1, HD], BF16, tag="ones1")
            nc.vector.memset(ones1[:], 1.0)
            ones11 = const.tile([65, 1], BF16, tag="ones11")
            nc.vector.memset(ones11[:], 1.0)

            if v_bias_nz:
                # broadcast the v-bias (free axis) across partitions via matmul
                bv_bf = const.tile([1, HDL], BF16, tag="bvbf")
                bvrow = const.tile([1, HDL], F32, tag="bvrow")
                for j in range(4):
                    nc.sync.dma_start(
                        bvrow[0:1, j * P : (j + 1) * P],
                        bqkv_sb[:, 8 + j : 9 + j].rearrange("p one -> one p"),
                    )
                nc.vector.tensor_copy(bv_bf[:], bvrow[:])
                ones128 = const.tile([1, P], BF16, tag="ones128")
                nc.vector.memset(ones128[:], 1.0)
                ps_bv = ps_s.tile([P, 2 * QW], F32, tag="s")
                nc.tensor.matmul(ps_bv[:, :HDL], ones128[:], bv_bf[:],
                                 start=True, stop=True)
                bv_bc = const.tile([P, HDL], F32, tag="bvbc")
                nc.vector.tensor_copy(bv_bc[:], ps_bv[:, :HDL])

            def qk_copy(dst_ap, ps_ap, bias_ap):
                if qk_bias_nz:
                    nc.vector.tensor_scalar_add(dst_ap, ps_ap, bias_ap)
                else:
                    nc.vector.tensor_copy(dst_ap, ps_ap)

            def alt_ps(i):
                """Alternate psum allocations between the two pools (4-deep
                rotation) so PSUM->SBUF copies never gate the next group."""
                pool, tag = ((ps_s, "s"), (ps_z, "z"))[i % 2]
                return pool.tile([P, 2 * QW], F32, tag=tag, name=f"ps_{tag}")

            _alt = [0]

            def fill_ps(state):
                if "ps" not in state:
                    pool, tag = state.get("pt", (ps_z, "z"))
                    state["ps"] = pool.tile([P, 2 * QW], F32, tag=tag,
                                            name="ps_fill")
                return state["ps"]

            def proj_group(tcA, fo, pt=None):
                """One feature-tile of Q/K projection for chunks tcA, tcA+1,
                as a list of micro-steps (matmuls + final copy)."""
                tok2 = slice(tcA * QW, (tcA + 2) * QW)
                fsl = slice(fo * P, (fo + 1) * P)
                state = {} if pt is None else {"pt": pt}
                steps = []

                def mk(kd, h):
                    def f():
                        ps = fill_ps(state)
                        nc.tensor.matmul(
                            ps[:, h * QW : (h + 1) * QW], wqkv_sb[:, kd, fsl],
                            xt_sb[:, kd, (tcA + h) * QW : (tcA + h + 1) * QW],
                            start=(kd == 0), stop=(kd == KD - 1),
                        )
                    return f

                for kd in range(KD):
                    steps.append(mk(kd, 0))
                    steps.append(mk(kd, 1))

                def fin():
                    ps = state["ps"]
                    if fo < 4:
                        qk_copy(qT[:, fo, tok2], ps[:], bqkv_sb[:, fo : fo + 1])
                    else:
                        qk_copy(kT[:, fo - 4, tok2], ps[:],
                                bqkv_sb[:, fo : fo + 1])

                steps.append(fin)
                return steps

            def v_group(tp, pt=None):
                """V projection for token tiles tp, tp+1 (token-major)."""
                state = {} if pt is None else {"pt": pt}
                steps = []

                def mk(kd, h):
                    def f():
                        ps = fill_ps(state)
                        nc.tensor.matmul(
                            ps[:, h * QW : (h + 1) * QW],
                            xt_sb[:, kd, (tp + h) * P : (tp + h + 1) * P],
                            wqkv_sb[:, kd, 2 * HDL : 3 * HDL],
                            start=(kd == 0), stop=(kd == KD - 1),
                        )
                    return f

                for kd in range(KD):
                    steps.append(mk(kd, 0))
                    steps.append(mk(kd, 1))

                def fin():
                    ps = state["ps"]
                    psv = ps[:].rearrange("p (t h d) -> p h t d", t=2, d=HD)
                    if v_bias_nz:
                        nc.vector.tensor_tensor(
                            v_sb[:, :, tp : tp + 2, 0:HD], psv,
                            bv_bc[:].rearrange("p (h d) -> p h d", d=HD)[
                                :, :, None, :
                            ].to_broadcast((P, LOC_H, 2, HD)),
                            mybir.AluOpType.add,
                        )
                    else:
                        nc.vector.tensor_copy(v_sb[:, :, tp : tp + 2, 0:HD], psv)

                steps.append(fin)
                return steps

            def proj_groups(tcA):
                return [proj_group(tcA, fo) for fo in range(8)] + [
                    v_group(tp) for tp in range(4 * tcA, 4 * (tcA + 2), 2)
                ]

            pend = []   # deferred normalize closures (keep <= 1)

            def attn_unit(qt, p_i, fast_rt=False, rt_eng=None):
                dmae = rt_eng or nc.sync
                nkt = 4 * (qt + 1)
                qs = slice(qt * QW, (qt + 1) * QW)
                psZ = ps_z.tile([P, 2 * QW], F32, tag="z")
                u_tiles = [None] * nkt

                def av(kt):
                    # diagonal tiles only touch queries >= m*128 (causal)
                    m = kt - 4 * qt
                    o = m * P if m > 0 else 0
                    first, last = (kt == 0), (kt == nkt - 1)
                    nc.tensor.matmul(
                        psZ[0 : HD + 1, o:QW], v_sb[:, 2 * p_i, kt, :],
                        u_tiles[kt][:, o:QW],
                        start=first, stop=last, skip_group_check=True,
                    )
                    nc.tensor.matmul(
                        psZ[0 : HD + 1, QW + o : 2 * QW], v_sb[:, 2 * p_i + 1, kt, :],
                        u_tiles[kt][:, QW + o : 2 * QW],
                        start=first, stop=last, skip_group_check=True,
                    )

                for kt in range(nkt):
                    ks = slice(kt * P, (kt + 1) * P)
                    m = kt - 4 * qt
                    o = m * P if m > 0 else 0
                    psS = ps_s.tile([P, 2 * QW], F32, tag="s")
                    nc.tensor.matmul(psS[:, o:QW], kT[0:64, p_i, ks],
                                     qT[0:64, p_i, qs][:, o:QW],
                                     start=True, stop=True)
                    nc.tensor.matmul(psS[:, QW + o : 2 * QW], kT[64:128, p_i, ks],
                                     qT[64:128, p_i, qs][:, o:QW],
                                     start=True, stop=True)
                    u = upool.tile([P, 2 * QW], BF16, tag="U")
                    u_tiles[kt] = u
                    if m < 0:
                        nc.scalar.activation(u[:], psS[:], FN.Exp)
                    else:
                        uv = u[:].rearrange("p (h q) -> p h q", h=2)
                        pv = psS[:].rearrange("p (h q) -> p h q", h=2)
                        nc.scalar.activation(
                            uv[:, :, o:QW], pv[:, :, o:QW], FN.Exp
                        )
                        blk = slice(o, o + P)
                        nc.vector.tensor_tensor(
                            uv[:, :, blk], uv[:, :, blk],
                            tri[:, None, :].to_broadcast((P, 2, P)), MUL,
                        )
                    if kt >= 2:
                        av(kt - 2)
                    pump(2)
                av(nkt - 2)
                av(nkt - 1)
                close_open_group()

                # part 1: evict z to SBUF (frees the PSUM bank), extract the
                # denominators, spread over 128 partitions, reciprocal, and
                # land in DRAM as [1024] bf16 rinv (A-half | B-half).
                # fast_rt (used for the last unit, where the latency is bare):
                # spread via tiny PE matmuls (r-row as weights) instead of the
                # two extra DMA hops through DRAM.
                if fast_rt:
                    stb = work4.tile([65, 2 * QW], BF16, tag="rstb")
                    nc.vector.tensor_copy(stb[64:65, :], psZ[64:65, :])
                    z_st = work4.tile([HD, 2 * QW], F32, tag="zst")
                    nc.vector.tensor_copy(z_st[:], psZ[0:HD, :])
                    psR2 = ps_s.tile([P, 2 * QW], F32, tag="s", name="ps_spread")
                    for j in range(8):
                        nc.tensor.matmul(psR2[:, j : j + 1],
                                         stb[64:65, j * P : (j + 1) * P],
                                         ones11[64:65, :], start=True, stop=True)
                    rspo = work4.tile([P, 8], BF16, tag="rspo")
                    with nc.allow_low_precision(reason="bf16 softmax denom"):
                        nc.vector.reciprocal(rspo[:], psR2[:, 0:8])
                    d2 = dram.tile([1, 2 * QW], BF16, tag="d2")
                    nc.sync.dma_start(
                        d2[:].rearrange("one (f p) -> (one p) f", p=P), rspo[:]
                    )
                else:
                    st = work4.tile([65, 2 * QW], F32, tag="rst")
                    nc.vector.tensor_copy(st[64:65, :], psZ[64:65, :])
                    z_st = work4.tile([HD, 2 * QW], F32, tag="zst")
                    nc.vector.tensor_copy(z_st[:], psZ[0:HD, :])
                    d1 = dram.tile([1, 2 * QW], F32, tag="d1")
                    dmae.dma_start(d1[:], st[64:65, :])
                    rsp = work4.tile([16, HD], F32, tag="rsp")
                    dmae.dma_start(
                        rsp[:], d1[:].rearrange("one (p f) -> (one p) f", p=16)
                    )
                    rspo = work4.tile([16, HD], BF16, tag="rspo")
                    with nc.allow_low_precision(reason="bf16 softmax denom"):
                        nc.vector.reciprocal(rspo[:], rsp[:])
                    d2 = dram.tile([1, 2 * QW], BF16, tag="d2")
                    dmae.dma_start(
                        d2[:].rearrange("one (p f) -> (one p) f", p=16), rspo[:]
                    )

                def part2():
                    # broadcast 1/r across partitions straight from DRAM
                    rb = work.tile([HD, 2 * QW], BF16, tag="rb")
                    for q in range(4):
                        dmae.dma_start(
                            rb[q * 16 : (q + 1) * 16, :],
                            d2[0:1, :].to_broadcast((16, 2 * QW)),
                        )
                    nc.vector.tensor_tensor(
                        zT[0:HD, p_i, qs], z_st[:, 0:QW], rb[:, 0:QW], MUL
                    )
                    stB = work.tile([HD, QW], BF16, tag="stB")
                    nc.vector.tensor_tensor(
                        stB[:], z_st[:, QW : 2 * QW], rb[:, QW : 2 * QW], MUL
                    )
                    dmae.dma_start(zT[64:128, p_i, qs], stB[:])

                return part2

            fillers = []      # list of group step-lists; groups[0] may be open
            fill_open = [False]

            def pump(n):
                done = 0
                while done < n and fillers:
                    g = fillers[0]
                    g.pop(0)()
                    fill_open[0] = True
                    done += 1
                    if not g:
                        fillers.pop(0)
                        fill_open[0] = False

            def close_open_group():
                if fillers and fill_open[0]:
                    g = fillers.pop(0)
                    for f in g:
                        f()
                    fill_open[0] = False

            def attn(qt, depth=3, fast_last=False):
                for p_i in range(NPAIR):
                    p2 = attn_unit(qt, p_i, fast_rt=(fast_last and p_i == 3),
                                   rt_eng=nc.scalar if (qt == 3 and p_i == 3)
                                   else None)
                    pend.append(p2)
                    while len(pend) > depth:
                        pend.pop(0)()

            def flush():
                close_open_group()
                while pend or fillers:
                    if pend:
                        pend.pop(0)()
                    if fillers:
                        for f in fillers.pop(0):
                            f()

            out_r = out_d[:].rearrange("(mo p) t -> p mo t", p=P)

            def op_group(qtA, mo, two):
                """out-projection group: query tiles qtA(,qtA+1), one mo tile."""
                w = 2 * QW if two else QW
                qs2 = slice(qtA * QW, qtA * QW + w)
                msl = slice(mo * P, (mo + 1) * P)
                state = {}
                steps = []

                def mk(ko, h):
                    def f():
                        psO = fill_ps(state)
                        qs = slice((qtA + h) * QW, (qtA + h + 1) * QW)
                        nc.tensor.matmul(psO[:, h * QW : (h + 1) * QW],
                                         wout_sb[:, ko, msl], zT[:, ko, qs],
                                         start=(ko == 0), stop=(ko == 3))
                    return f

                for ko in range(4):
                    for h in range(2 if two else 1):
                        steps.append(mk(ko, h))

                def fin():
                    psO = state["ps"]
                    osb = work.tile([P, 2 * QW], F32, tag="osb")
                    if out_bias_nz:
                        nc.vector.tensor_scalar_add(osb[:, 0:w], psO[:, 0:w],
                                                    bout_sb[:, mo : mo + 1])
                    else:
                        nc.vector.tensor_copy(osb[:, 0:w], psO[:, 0:w])
                    nc.sync.dma_start(out_r[:, mo, qs2], osb[:, 0:w])

                steps.append(fin)
                return steps

            def op_groups(qtA, two=True):
                return [op_group(qtA, mo, two) for mo in range(8)]

            # ---- schedule: later projection / out-projection groups are
            # micro-woven into attention (<=2 filler matmuls per ktile
            # iteration: attention is ACT-paced, the fillers use the idle
            # TensorEngine slots without starving the exp pipeline) ----
            # proj01 runs kd-outer in batches of 4 concurrent psum groups so
            # the first matmuls only need the first input DMA pieces (the PE
            # ramps with the loads instead of idling ~20us).
            pts = [(ps_s, "s"), (ps_s, "s"), (ps_z, "z"), (ps_z, "z")]
            p01_sets = [
                [proj_group(0, fo, pts[i]) for i, fo in enumerate(range(0, 4))],
                [proj_group(0, fo, pts[i]) for i, fo in enumerate(range(4, 8))],
                [v_group(tp, pts[i]) for i, tp in enumerate(range(0, 8, 2))],
            ]
            for gset in p01_sets:
                n = len(gset[0]) - 1
                for i in range(n):
                    for g in gset:
                        g[i]()
                for g in gset:
                    g[n]()
            fillers.extend(proj_groups(2))       # 12 groups
            attn(0)
            attn(1)
            flush()                               # qt0/qt1 normalizes done
            fillers.extend(op_groups(0, two=True))
            attn(2)
            flush()
            op2 = op_groups(2, two=False)
            fillers.extend(op2[:2])
            attn(3, depth=1)
            close_open_group()
            for g in fillers:
                for f in g:
                    f()
            fillers.clear()
            for g in op2[2:]:
                for f in g:
                    f()
            # tail: compute out-proj(qt3) contributions of pairs 0-2 while the
            # final unit's reciprocal round-trip is in flight, then finish
            # with pair 3 once its normalize lands.
            qs3 = slice(3 * QW, 4 * QW)
            tails = []
            for mh in range(4):
                psO = pts[mh][0].tile([P, 2 * QW], F32, tag=pts[mh][1],
                                      name="ps_tail")
                for h in range(2):
                    mo = 2 * mh + h
                    msl = slice(mo * P, (mo + 1) * P)
                    for ko in range(3):
                        nc.tensor.matmul(psO[:, h * QW : (h + 1) * QW],
                                         wout_sb[:, ko, msl], zT[:, ko, qs3],
                                         start=(ko == 0), stop=False,
                                         skip_group_check=True)
                tails.append(psO)
            flush()   # part2(qt3, p3)
            for mh in range(4):
                psO = tails[mh]
                for h in range(2):
                    mo = 2 * mh + h
                    msl = slice(mo * P, (mo + 1) * P)
                    nc.tensor.matmul(psO[:, h * QW : (h + 1) * QW],
                                     wout_sb[:, 3, msl], zT[:, 3, qs3],
                                     start=False, stop=True,
                                     skip_group_check=True)
                osb = work.tile([P, 2 * QW], F32, tag="osb")
                if out_bias_nz:
                    for h in range(2):
                        mo = 2 * mh + h
                        nc.vector.tensor_scalar_add(
                            osb[:, h * QW : (h + 1) * QW],
                            psO[:, h * QW : (h + 1) * QW],
                            bout_sb[:, mo : mo + 1])
                else:
                    nc.vector.tensor_copy(osb[:], psO[:])
                for h in range(2):
                    mo = 2 * mh + h
                    nc.sync.dma_start(out_r[:, mo, qs3],
                                      osb[:, h * QW : (h + 1) * QW])

    nc.finalize()
    return nc


def _tile_p(a, inner):
    """[n*128, m...] -> [128, n, m...] partition-major, contiguous."""
    n = a.shape[0] // P
    return np.ascontiguousarray(
        a.reshape(n, P, *a.shape[1:]).transpose(1, 0, *range(2, a.ndim + 1))
    )


def kernel(x, w_qkv, b_qkv, w_out, b_out):
    global LAST_RESULT
    x = np.asarray(x)
    w_qkv = np.asarray(w_qkv, dtype=np.float32)
    b_qkv = np.asarray(b_qkv, dtype=np.float32)
    w_out = np.asarray(w_out, dtype=np.float32)
    b_out = np.asarray(b_out, dtype=np.float32)
    B = x.shape[0]

    in_maps = []
    qk_bias_nz = bool(np.any(b_qkv[: 2 * D] != 0.0))
    v_bias_nz = bool(np.any(b_qkv[2 * D :] != 0.0))
    out_bias_nz = bool(np.any(b_out != 0.0))
    for c in range(N_CORES):
        b = c // 2
        hg = c % 2
        heads = range(hg * LOC_H, (hg + 1) * LOC_H)
        cols = np.array(
            [sec * D + h * HD + j for sec in range(3) for h in heads
             for j in range(HD)]
        )
        w_loc = w_qkv[:, cols].copy()
        w_loc[:, HDL : 2 * HDL] *= 1.0 / np.sqrt(HD)
        b_loc = b_qkv[cols].copy()
        b_loc[HDL : 2 * HDL] *= 1.0 / np.sqrt(HD)
        bo = b_out if hg == 0 else np.zeros_like(b_out)
        xt = np.ascontiguousarray(x[b].T)
        in_maps.append(
            dict(
                xt=_tile_p(xt.astype(BF), KD),
                wqkv=_tile_p(w_loc.astype(BF), KD),
                bqkv=np.ascontiguousarray(b_loc.reshape(12, P).T),
                wout=_tile_p(w_out[cols[2 * HDL :] - 2 * D, :].astype(BF), 4),
                bout=np.ascontiguousarray(bo.reshape(8, P).T),
            )
        )

    key = (qk_bias_nz, v_bias_nz, out_bias_nz)
    if key not in _NC_CACHE:
        _NC_CACHE[key] = build_nc(*key)
    nc = _NC_CACHE[key]

    res = run_bass_kernel_spmd(
        nc, in_maps, core_ids=list(range(N_CORES)), trace=TRACE
    )
    LAST_RESULT = res

    out = np.empty((B, S, D), dtype=np.float32)
    for b in range(B):
        out[b] = (res.results[2 * b]["out"] + res.results[2 * b + 1]["out"]).T
    return out
